# revision 8
# baseline (speedup 1.0000x reference)
"""TRN2 Bass kernel for nn_CausalSelfAttention_63058709840004.

Sharding: tensor-parallel over heads (2 groups x 3 heads) x 4 causal query
shards = 8 cores. Each core computes K,V for its 3 heads over the full
sequence (replicated within the group), Q for its 1024 query rows (two
512-row chunks at s*512 and (s+4)*512), runs causal attention, and a partial
c_proj; the host sums the two head-group partials per row.

All matmuls run as float32r (TF32-like, 1 cyc/row at N>=256, ~1.6e-4 rel
err). rms-norm + rotary are done in natural [t, d] layout (free-axis
reductions + per-partition scales), then Q/K are PE-transposed to [d, t] for
the attention matmuls. Softmax skips max-subtraction (|scores| <= 15.4
bounded by Cauchy-Schwarz after rms-norm, safe in fp32).
"""
import numpy as np

import concourse.bass as bass
import concourse.bacc as bacc
import concourse.mybir as mybir
import concourse.tile as tile
from concourse.bass_utils import run_bass_kernel_spmd

T, DIM, H, D = 4096, 768, 6, 128
HPG = 3  # heads per group
GDIM = HPG * D  # 384
ATTN_SCALE = 0.12
EPS = 1.1920929e-07
NT = T // 128  # 32 t-tiles
NQ = 1024 // 128  # 8 q-tiles per core
F32 = mybir.dt.float32
F32R = mybir.dt.float32r
U32 = mybir.dt.uint32
EXP = mybir.ActivationFunctionType.Exp
SQUARE = mybir.ActivationFunctionType.Square
SQRT = mybir.ActivationFunctionType.Sqrt
MASK_NEG = -1.0e5

_CACHE = {}


def _rotary(nc, pool, nat, cos_b, sin_p, sin_n, nh):
    """In-place rotary on nat [128, nh, 128] (scaled). Rotates dim pairs
    (i, 64+i) for i in 0..31 (freqs 32..63 are zero -> identity)."""
    x1 = nat[:, :, 0:32]
    x2 = nat[:, :, 64:96]
    ta = pool.tile([128, nh, 32], F32R, tag="rot_ta")
    tb = pool.tile([128, nh, 32], F32R, tag="rot_tb")
    ua = pool.tile([128, nh, 32], F32R, tag="rot_ua")
    ub = pool.tile([128, nh, 32], F32R, tag="rot_ub")
    nc.vector.tensor_mul(out=ta[:], in0=x2, in1=sin_p)  # x2*sin
    nc.vector.tensor_mul(out=tb[:], in0=x1, in1=sin_n)  # -x1*sin
    nc.vector.tensor_mul(out=ua[:], in0=x1, in1=cos_b)  # x1*cos
    nc.vector.tensor_mul(out=ub[:], in0=x2, in1=cos_b)  # x2*cos
    nc.vector.tensor_add(out=x1, in0=ua[:], in1=ta[:])  # y1 = x1*c + x2*s
    nc.vector.tensor_add(out=x2, in0=ub[:], in1=tb[:])  # y2 = x2*c - x1*s


def build_nc():
    nc = bacc.Bacc(None, target_bir_lowering=False)

    # ---- DRAM tensors (per-core inputs prepared by the host) ----
    xT = nc.dram_tensor("xT", [DIM, T], F32R, kind="ExternalInput")
    xqT = nc.dram_tensor("xqT", [DIM, 1024], F32R, kind="ExternalInput")
    wkv = nc.dram_tensor("wkv", [DIM, 2 * GDIM], F32R, kind="ExternalInput")
    wq = nc.dram_tensor("wq", [DIM, GDIM], F32R, kind="ExternalInput")
    vek = nc.dram_tensor("vek", [T, GDIM], F32, kind="ExternalInput")
    cosk = nc.dram_tensor("cosk", [T, 32], F32, kind="ExternalInput")
    sinkpm = nc.dram_tensor("sinkpm", [T, 64], F32, kind="ExternalInput")
    cosq = nc.dram_tensor("cosq", [1024, 32], F32, kind="ExternalInput")
    sinqpm = nc.dram_tensor("sinqpm", [1024, 64], F32, kind="ExternalInput")
    cprojT = nc.dram_tensor("cprojT", [GDIM, DIM], F32R, kind="ExternalInput")
    maskd = nc.dram_tensor("maskd", [4, 128, 512], F32, kind="ExternalInput")
    ident_in = nc.dram_tensor("ident", [128, 128], F32R, kind="ExternalInput")
    ones_col_in = nc.dram_tensor("ones_col", [128, 1], F32R, kind="ExternalInput")
    ones_row_in = nc.dram_tensor("ones_row", [1, 128], F32R, kind="ExternalInput")
    svar_t = nc.dram_tensor("svar", [1, 1], U32, kind="ExternalInput")
    y_out = nc.dram_tensor("y", [1024, DIM], F32, kind="ExternalOutput")

    xT_r = xT.rearrange("(o p) t -> p o t", p=128)
    xqT_r = xqT.rearrange("(o p) t -> p o t", p=128)

    with tile.TileContext(nc) as tc:
        # core-variant register (s = core % 4)
        tmp = nc.alloc_registers("tmp_svar", mybir.ALL_ENGINES)
        nc.regs_load(tmp, svar_t[0:1, 0:1])
        sv = nc.snap(tmp, donate=True, min_val=0, max_val=3)

        with tc.tile_pool(name="res", bufs=1) as res:
            KT = res.tile([128, HPG, T], F32R, tag="KT")
            Vn = res.tile([128, NT, GDIM], F32R, tag="Vn")
            QT = res.tile([128, HPG, 1024], F32R, tag="QT")
            Ysb = res.tile([128, HPG, 1024], F32R, tag="Ysb")
            cproj_sb = res.tile([128, HPG, DIM], F32R, tag="cproj")
            maskd_sb = res.tile([128, 4, 512], F32, tag="maskd")
            ident = res.tile([128, 128], F32R, tag="ident")
            ones_col = res.tile([128, 1], F32R, tag="ones_col")
            ones_row = res.tile([1, 128], F32R, tag="ones_row")
            nc.sync.dma_start(cproj_sb[:], cprojT.rearrange("(o p) d -> p o d", p=128))
            nc.sync.dma_start(maskd_sb[:], maskd.rearrange("i p j -> p i j"))
            nc.sync.dma_start(ident[:], ident_in[:])
            nc.sync.dma_start(ones_col[:], ones_col_in[:])
            nc.sync.dma_start(ones_row[:], ones_row_in[:])
            eps_k = res.tile([128, 1], F32, tag="eps_k")
            eps_q = res.tile([128, 1], F32, tag="eps_q")
            nc.gpsimd.memset(eps_k[:], EPS)
            nc.gpsimd.memset(eps_q[:], EPS / (ATTN_SCALE * ATTN_SCALE))

            # ================= Phase A/B: projections =================
            with (
                tc.tile_pool(name="wp", bufs=1) as wp,
                tc.tile_pool(name="ap", bufs=3) as ap,
                tc.tile_pool(name="rot", bufs=2) as rot,
                tc.tile_pool(name="pp", bufs=3, space="PSUM") as pp,
                tc.tile_pool(name="pt", bufs=2, space="PSUM") as pt,
            ):
                wkv_sb = wp.tile([128, 6, 2 * GDIM], F32R, tag="wkv")
                wq_sb = wp.tile([128, 6, GDIM], F32R, tag="wq")
                cosk_sb = wp.tile([128, NT, 32], F32, tag="cosk")
                sinkpm_sb = wp.tile([128, NT, 64], F32, tag="sinkpm")
                cosq_sb = wp.tile([128, NQ, 32], F32, tag="cosq")
                sinqpm_sb = wp.tile([128, NQ, 64], F32, tag="sinqpm")
                nc.sync.dma_start(wkv_sb[:], wkv.rearrange("(o p) d -> p o d", p=128))
                nc.sync.dma_start(wq_sb[:], wq.rearrange("(o p) d -> p o d", p=128))
                nc.sync.dma_start(cosk_sb[:], cosk.rearrange("(n p) c -> p n c", p=128))
                nc.sync.dma_start(sinkpm_sb[:], sinkpm.rearrange("(n p) c -> p n c", p=128))
                nc.sync.dma_start(cosq_sb[:], cosq.rearrange("(n p) c -> p n c", p=128))
                nc.sync.dma_start(sinqpm_sb[:], sinqpm.rearrange("(n p) c -> p n c", p=128))

                # ---- Phase A: K,V over full sequence ----
                for ti in range(NT):
                    xt = ap.tile([128, 6, 128], F32R, tag="xt")
                    nc.sync.dma_start(xt[:], xT_r[:, :, ti * 128 : (ti + 1) * 128])
                    vet = ap.tile([128, GDIM], F32, tag="vet")
                    nc.sync.dma_start(vet[:], vek[ti * 128 : (ti + 1) * 128, :])
                    k_ps = pp.tile([128, GDIM], F32, tag="k_ps")
                    v_ps = pp.tile([128, GDIM], F32, tag="v_ps")
                    for md in range(6):
                        nc.tensor.matmul(
                            k_ps[:], xt[:, md], wkv_sb[:, md, 0:GDIM],
                            start=(md == 0), stop=(md == 5), skip_group_check=True,
                        )
                        nc.tensor.matmul(
                            v_ps[:], xt[:, md], wkv_sb[:, md, GDIM : 2 * GDIM],
                            start=(md == 0), stop=(md == 5), skip_group_check=True,
                        )
                    # V: add pre-scaled ve, store natural
                    nc.vector.tensor_add(out=Vn[:, ti, :], in0=v_ps[:], in1=vet[:])
                    # K: rms-norm scale b = 1/sqrt(mean(k^2)+eps) per row/head
                    ssq = ap.tile([128, HPG], F32, tag="ssq")
                    scratch = ap.tile([128, GDIM], F32, tag="scratch")
                    for h in range(HPG):
                        nc.scalar.activation(
                            scratch[:, h * D : (h + 1) * D], k_ps[:, h * D : (h + 1) * D],
                            SQUARE, accum_out=ssq[:, h : h + 1],
                        )
                    bsc = ap.tile([128, HPG], F32, tag="bsc")
                    nc.scalar.activation(bsc[:], ssq[:], SQRT, bias=eps_k[:], scale=1.0 / D)
                    nc.vector.reciprocal(bsc[:], bsc[:])
                    knat = ap.tile([128, HPG, D], F32R, tag="knat")
                    for h in range(HPG):
                        nc.scalar.activation(
                            knat[:, h], k_ps[:, h * D : (h + 1) * D],
                            mybir.ActivationFunctionType.Copy, scale=bsc[:, h : h + 1],
                        )
                    _rotary(
                        nc, rot, knat,
                        cosk_sb[:, ti, None, :].to_broadcast((128, HPG, 32)),
                        sinkpm_sb[:, ti, None, 0:32].to_broadcast((128, HPG, 32)),
                        sinkpm_sb[:, ti, None, 32:64].to_broadcast((128, HPG, 32)),
                        HPG,
                    )
                    for h in range(HPG):
                        tr = pt.tile([128, 128], F32R, tag="tr")
                        nc.tensor.transpose(tr[:], knat[:, h], ident[:])
                        nc.vector.tensor_copy(KT[:, h, ti * 128 : (ti + 1) * 128], tr[:])

                # ---- Phase B: Q over this core's 1024 rows ----
                for ti in range(NQ):
                    xt = ap.tile([128, 6, 128], F32R, tag="xt")
                    nc.sync.dma_start(xt[:], xqT_r[:, :, ti * 128 : (ti + 1) * 128])
                    q_ps = pp.tile([128, GDIM], F32, tag="k_ps")
                    for md in range(6):
                        nc.tensor.matmul(
                            q_ps[:], xt[:, md], wq_sb[:, md],
                            start=(md == 0), stop=(md == 5), skip_group_check=True,
                        )
                    ssq = ap.tile([128, HPG], F32, tag="ssq")
                    scratch = ap.tile([128, GDIM], F32, tag="scratch")
                    for h in range(HPG):
                        nc.scalar.activation(
                            scratch[:, h * D : (h + 1) * D], q_ps[:, h * D : (h + 1) * D],
                            SQUARE, accum_out=ssq[:, h : h + 1],
                        )
                    # a = ATTN_SCALE / sqrt(mean+eps) = 1/sqrt((m/D+eps)/s^2)
                    asc = ap.tile([128, HPG], F32, tag="bsc")
                    s2 = ATTN_SCALE * ATTN_SCALE
                    nc.scalar.activation(asc[:], ssq[:], SQRT, bias=eps_q[:], scale=1.0 / (D * s2))
                    nc.vector.reciprocal(asc[:], asc[:])
                    qnat = ap.tile([128, HPG, D], F32R, tag="knat")
                    for h in range(HPG):
                        nc.scalar.activation(
                            qnat[:, h], q_ps[:, h * D : (h + 1) * D],
                            mybir.ActivationFunctionType.Copy, scale=asc[:, h : h + 1],
                        )
                    _rotary(
                        nc, rot, qnat,
                        cosq_sb[:, ti, None, :].to_broadcast((128, HPG, 32)),
                        sinqpm_sb[:, ti, None, 0:32].to_broadcast((128, HPG, 32)),
                        sinqpm_sb[:, ti, None, 32:64].to_broadcast((128, HPG, 32)),
                        HPG,
                    )
                    for h in range(HPG):
                        tr = pt.tile([128, 128], F32R, tag="tr")
                        nc.tensor.transpose(tr[:], qnat[:, h], ident[:])
                        nc.vector.tensor_copy(QT[:, h, ti * 128 : (ti + 1) * 128], tr[:])

            # ================= Phase C: attention (per-core variant) ======
            def attention(s):
                with (
                    tc.tile_pool(name=f"ep{s}", bufs=6) as ep,
                    tc.tile_pool(name=f"rp{s}", bufs=2) as rp,
                    tc.tile_pool(name=f"psS{s}", bufs=3, space="PSUM") as psS,
                    tc.tile_pool(name=f"psY{s}", bufs=2, space="PSUM") as psY,
                    tc.tile_pool(name=f"psD{s}", bufs=2, space="PSUM") as psD,
                ):
                    chunks = [(0, 4 * (s + 1)), (512, 4 * (s + 5))]
                    for h in range(HPG):
                        for qoff, nk in chunks:
                            y_ps = psY.tile([128, 512], F32, tag="y")
                            d_ps = psD.tile([1, 512], F32, tag="d")
                            for kb in range(0, nk, 4):
                                kg = list(range(kb, min(kb + 4, nk)))
                                Es = []
                                for k in kg:
                                    s_ps = psS.tile([128, 512], F32, tag="s")
                                    nc.tensor.matmul(
                                        s_ps[:],
                                        KT[:, h, k * 128 : (k + 1) * 128],
                                        QT[:, h, qoff : qoff + 512],
                                        start=True, stop=True, skip_group_check=True,
                                    )
                                    i = k - (nk - 4)
                                    if i >= 0:
                                        nc.vector.tensor_add(
                                            out=s_ps[:], in0=s_ps[:], in1=maskd_sb[:, i]
                                        )
                                    E = ep.tile([128, 512], F32R, tag="E")
                                    nc.scalar.activation(E[:], s_ps[:], EXP)
                                    Es.append(E)
                                for j, E in zip(kg, Es):
                                    nc.tensor.matmul(
                                        d_ps[:], ones_col[:], E[:],
                                        start=(j == 0), stop=(j == nk - 1),
                                        skip_group_check=True,
                                    )
                                for j, E in zip(kg, Es):
                                    nc.tensor.matmul(
                                        y_ps[:], Vn[:, j, h * D : (h + 1) * D], E[:],
                                        start=(j == 0), stop=(j == nk - 1),
                                        skip_group_check=True,
                                    )
                            recip = rp.tile([1, 512], F32R, tag="recip")
                            with nc.allow_low_precision(
                                reason="1/denom as f32r for matmul broadcast; ~1e-4 uniform scale wobble"
                            ):
                                nc.vector.reciprocal(recip[:], d_ps[:])
                            bc = psS.tile([128, 512], F32, tag="s")
                            nc.tensor.matmul(
                                bc[:], ones_row[:], recip[:],
                                start=True, stop=True, skip_group_check=True,
                            )
                            ysl = Ysb[:, h, qoff : qoff + 512]
                            nc.vector.tensor_copy(ysl, y_ps[:])
                            nc.vector.tensor_mul(out=ysl, in0=ysl, in1=bc[:])

            with tc.If(sv == 0) as c0:
                attention(0)
            with c0.Else():
                with tc.If(sv == 1) as c1:
                    attention(1)
                with c1.Else():
                    with tc.If(sv == 2) as c2:
                        attention(2)
                    with c2.Else():
                        attention(3)

            # ================= Phase D: c_proj partial ====================
            with (
                tc.tile_pool(name="op", bufs=3) as op,
                tc.tile_pool(name="psO", bufs=3, space="PSUM") as psO,
            ):
                for m in range(NQ):
                    o_sb = op.tile([128, DIM], F32, tag="o_sb")
                    for oc in range(2):
                        o_ps = psO.tile([128, GDIM], F32, tag="o_ps")
                        for h in range(HPG):
                            nc.tensor.matmul(
                                o_ps[:],
                                Ysb[:, h, m * 128 : (m + 1) * 128],
                                cproj_sb[:, h, oc * GDIM : (oc + 1) * GDIM],
                                start=(h == 0), stop=(h == 2), skip_group_check=True,
                            )
                        nc.scalar.copy(o_sb[:, oc * GDIM : (oc + 1) * GDIM], o_ps[:])
                    nc.sync.dma_start(y_out[m * 128 : (m + 1) * 128, :], o_sb[:])

    nc.finalize()
    return nc


def _host_prep(x, ve, qkv_w, lambdas, c_proj_w):
    """Build the 8 per-core input maps."""
    x2d = np.ascontiguousarray(x.reshape(T, DIM), dtype=np.float32)
    xT = np.ascontiguousarray(x2d.T)
    ve2 = ve.reshape(T, H, D).astype(np.float32)
    lam0, lam1 = float(lambdas[0]), float(lambdas[1])
    wq_all, wk_all, wv_all = qkv_w[0], qkv_w[1], qkv_w[2]  # [768, 768] each

    t = np.arange(T, dtype=np.float32)
    af = (1.0 / 1024.0) ** np.linspace(0.0, 1.0, 32, dtype=np.float32)
    theta = t[:, None] * af[None, :]
    cos_t = np.cos(theta).astype(np.float32)  # [T, 32]
    sin_t = np.sin(theta).astype(np.float32)
    sin_pm = np.concatenate([sin_t, -sin_t], axis=1)  # [T, 64]

    # diagonal triangle masks, shared across cores/chunks
    r = np.arange(128)[:, None]
    j = np.arange(512)[None, :]
    maskd = np.stack(
        [np.where(128 * i + r <= j, 0.0, MASK_NEG) for i in range(4)]
    ).astype(np.float32)

    ident = np.eye(128, dtype=np.float32)
    ones_col = np.ones((128, 1), dtype=np.float32)
    ones_row = np.ones((1, 128), dtype=np.float32)

    in_maps = []
    for c in range(8):
        g, s = divmod(c, 4)
        hsl = slice(g * GDIM, (g + 1) * GDIM)
        qrows = np.r_[512 * s : 512 * (s + 1), 512 * (s + 4) : 512 * (s + 5)]
        wkv = np.concatenate([wk_all[hsl], lam0 * wv_all[hsl]], axis=0)  # [768, 768]
        in_maps.append(
            {
                "xT": xT,
                "xqT": np.ascontiguousarray(xT[:, qrows]),
                "wkv": np.ascontiguousarray(wkv.T.astype(np.float32)),
                "wq": np.ascontiguousarray(wq_all[hsl].T.astype(np.float32)),
                "vek": np.ascontiguousarray(
                    (lam1 * ve2[:, g * HPG : (g + 1) * HPG, :]).reshape(T, GDIM)
                ),
                "cosk": cos_t,
                "sinkpm": sin_pm,
                "cosq": np.ascontiguousarray(cos_t[qrows]),
                "sinqpm": np.ascontiguousarray(sin_pm[qrows]),
                "cprojT": np.ascontiguousarray(c_proj_w[:, hsl].T.astype(np.float32)),
                "maskd": maskd,
                "ident": ident,
                "ones_col": ones_col,
                "ones_row": ones_row,
                "svar": np.array([[s]], dtype=np.uint32),
            }
        )
    return in_maps


def run(inputs, **run_kwargs):
    if "nc" not in _CACHE:
        _CACHE["nc"] = build_nc()
    nc = _CACHE["nc"]
    in_maps = _host_prep(
        inputs["x"], inputs["ve"], inputs["qkv_w"], inputs["lambdas"], inputs["c_proj_w"]
    )
    res = run_bass_kernel_spmd(nc, in_maps, core_ids=list(range(8)), **run_kwargs)
    out = np.zeros((T, DIM), dtype=np.float32)
    for c, r in enumerate(res.results):
        s = c % 4
        y = r["y"]
        out[512 * s : 512 * (s + 1)] += y[:512]
        out[512 * (s + 4) : 512 * (s + 5)] += y[512:]
    return out.reshape(1, T, DIM), res


def kernel(**inputs):
    out, _ = run(inputs)
    return out
